# revision 1
# baseline (speedup 1.0000x reference)
"""Trainium2 Bass kernel for a 2-layer GCN (nn_CorrelationGNN).

Math (reference):
    src,dst = edges + self loops;  deg over dst;  dinv = deg^-1/2
    h1 = relu(S @ (x @ W0) + b0),  S = D^-1/2 (A+I) D^-1/2
    h2 = relu(S @ (h1 @ W1) + b1)
    out = h2 @ Wf + bf

Factorization used: S @ (h W) = dinv * Agg(dinv * h) @ W, where Agg is the
pure 0/1 adjacency gather-sum (S commutes with the feature matmul).

Distribution: destination nodes sharded across 8 cores (12500/core, padded
to 12544 = 128*98).  Ranks are degree-sorted; rank r -> (p=r%128, g=r//128),
table row within a core slice = p*98+g.  Gather source is an fp16 table
[100352, 128] (row = 32 feats + 96 zeros = 256B) assembled per core from an
AllGather of compact fp16 slices.  Edges are gathered with gpsimd dma_gather
(int16 idxs, 4 SWDGE queues, <=1024 idxs/inst) as 4 source-quarter streams;
per (quarter, g-column) the slot count K is the max over all cores so the
traced program is identical on every core (SPMD).
"""

import numpy as np

import concourse.bass as bass  # noqa: F401
import concourse.bacc as bacc
import concourse.mybir as mybir
from concourse.tile import TileContext
from concourse.bass_utils import run_bass_kernel_spmd

P = 128
N = 100000
F = 32
NPC = 12500          # real nodes per core
G = 98               # g-columns per core
NPCP = P * G         # padded nodes per core = 12544
NROWS = 8 * NPCP     # global table rows = 100352
QROWS = NROWS // 4   # 25088, int16-addressable quarter
QZREL = 12543        # guaranteed-zero pad row, same offset in every quarter
KCAP = 8             # slots per dma_gather inst (8*128 = 1024 idx cap)
FDT = mybir.dt.float32
HDT = mybir.dt.float16


def _build_plan_and_offsets(edge_index):
    src = np.asarray(edge_index[0], dtype=np.int64)
    dst = np.asarray(edge_index[1], dtype=np.int64)
    loops = np.arange(N, dtype=np.int64)
    src = np.concatenate([src, loops])
    dst = np.concatenate([dst, loops])

    deg = np.bincount(dst, minlength=N).astype(np.float64)
    dinv = (1.0 / np.sqrt(deg)).astype(np.float32)

    node_core = np.arange(N) // NPC
    rank = np.empty(N, dtype=np.int64)
    perms = []
    for c in range(8):
        nodes = np.arange(c * NPC, (c + 1) * NPC)
        order = np.argsort(-deg[nodes], kind="stable")
        perm = nodes[order]
        perms.append(perm)
        rank[perm] = np.arange(NPC)
    trow = node_core * NPCP + (rank % P) * G + (rank // P)
    quarter = trow // QROWS
    qrel = (trow % QROWS).astype(np.int32)

    # per-core edges sorted by (dst rank, src quarter); self-loops handled
    # on-device as agg init = xd_own, so drop them from the token streams
    noloop = src != dst
    srcn, dstn = src[noloop], dst[noloop]
    edges = []
    cnt_rq = np.zeros((8, NPC * 4), dtype=np.int32)
    for c in range(8):
        m = (dstn // NPC) == c
        s, d = srcn[m], dstn[m]
        key = rank[d] * 4 + quarter[s]
        order = np.argsort(key, kind="stable")
        edges.append((qrel[s][order], key[order]))
        cnt_rq[c] = np.bincount(key, minlength=NPC * 4)

    crq = cnt_rq.reshape(8, NPC, 4)
    K = np.zeros((G, 4), dtype=np.int32)
    for g in range(G):
        K[g] = crq[:, g * P : (g + 1) * P, :].max(axis=(0, 1))

    # shared instruction plan: (q, g, k0, kc, col0)
    plan = []
    col = 0
    for q in range(4):
        for g in range(G):
            k0 = 0
            while k0 < int(K[g, q]):
                kc = min(KCAP, int(K[g, q]) - k0)
                plan.append((q, g, k0, kc, col))
                col += kc * P // 16
                k0 += kc
    totc = col

    # zero pad rows (pad ranks 12500.. of the two cores in each quarter have
    # dinv=0 so their table rows are always zero); spread pad tokens across
    # them to avoid HBM hot-spotting on a single row.
    pad_ranks = np.arange(NPC, NPCP)
    zrel = (pad_ranks % P) * G + (pad_ranks // P)  # within-slice rows
    zero_rows = np.concatenate([zrel, zrel + NPCP]).astype(np.int16)  # both cores

    # per-core offset tables [16, totc], idx i of inst at [i%16, col0+i//16]
    offs_all = []
    for c in range(8):
        qr, key = edges[c]
        cnt = np.bincount(key, minlength=NPC * 4)
        ptr = np.zeros(NPC * 4 + 1, dtype=np.int64)
        np.cumsum(cnt, out=ptr[1:])
        rngpad = np.random.default_rng(c)
        offs = zero_rows[rngpad.integers(0, len(zero_rows), size=totc * 16)].astype(
            np.int16
        )
        for (q, g, k0, kc, col0) in plan:
            base = col0 * 16
            lo = g * P
            nreal = min(P, NPC - lo)
            # vectorized over p
            ps = np.arange(nreal)
            rk = lo + ps
            a = ptr[rk * 4 + q]
            b = ptr[rk * 4 + q + 1]
            for kk in range(kc):
                k = k0 + kk
                sel = (a + k) < b
                pos = base + kk * P + ps[sel]
                offs[pos] = qr[(a + k)[sel]]
        offs_all.append(offs.reshape(totc, 16).T.copy())

    return plan, totc, dinv, perms, offs_all


def _build_program(plan, totc):
    nc = bacc.Bacc(
        "TRN2", target_bir_lowering=False, debug=False, num_devices=8,
        num_swdge_queues=4,
    )
    x_own = nc.declare_dram_parameter("x_own", [P, G * F], FDT, isOutput=False)
    dinv_own = nc.declare_dram_parameter("dinv_own", [P, G], FDT, isOutput=False)
    offs = nc.declare_dram_parameter("offs", [P, totc], mybir.dt.int16, isOutput=False)
    W0 = nc.declare_dram_parameter("W0", [F, F], FDT, isOutput=False)
    W1 = nc.declare_dram_parameter("W1", [F, F], FDT, isOutput=False)
    Wf = nc.declare_dram_parameter("Wf", [F, F], FDT, isOutput=False)
    b0 = nc.declare_dram_parameter("b0", [F, 1], FDT, isOutput=False)
    b1 = nc.declare_dram_parameter("b1", [F, 1], FDT, isOutput=False)
    bf = nc.declare_dram_parameter("bf", [F, 1], FDT, isOutput=False)
    out_own = nc.declare_dram_parameter("out_own", [P, G * F], FDT, isOutput=True)

    cc_in = nc.dram_tensor("cc_in", [NPCP, F], HDT)
    cc_out = nc.dram_tensor("cc_out", [NROWS, F], HDT, addr_space="Shared")
    table = nc.dram_tensor("table", [NROWS, 4 * F], HDT)

    from concourse.masks import make_identity

    with TileContext(nc) as tc:
        with (
            tc.tile_pool(name="persist", bufs=1) as pp,
            tc.tile_pool(name="gpool", bufs=10) as gp,
            tc.tile_pool(name="spool", bufs=4) as sp,
            tc.tile_pool(name="psum", bufs=2, space="PSUM") as psp,
        ):
            offs_t = pp.tile([P, totc], mybir.dt.int16)
            nc.sync.dma_start(out=offs_t[:], in_=offs[:])
            dinv_t = pp.tile([P, G], FDT)
            nc.sync.dma_start(out=dinv_t[:], in_=dinv_own[:])
            w0_t = pp.tile([F, F], FDT)
            nc.sync.dma_start(out=w0_t[:], in_=W0[:])
            w1_t = pp.tile([F, F], FDT)
            nc.sync.dma_start(out=w1_t[:], in_=W1[:])
            wf_t = pp.tile([F, F], FDT)
            nc.sync.dma_start(out=wf_t[:], in_=Wf[:])
            b0_t = pp.tile([F, 1], FDT)
            nc.sync.dma_start(out=b0_t[:], in_=b0[:])
            b1_t = pp.tile([F, 1], FDT)
            nc.sync.dma_start(out=b1_t[:], in_=b1[:])
            bf_t = pp.tile([F, 1], FDT)
            nc.sync.dma_start(out=bf_t[:], in_=bf[:])
            ident = pp.tile([P, P], FDT)
            make_identity(nc, ident[:])

            xcur = pp.tile([P, G * F], FDT, tag="xcur")
            nc.sync.dma_start(out=xcur[:], in_=x_own[:])
            agg = pp.tile([P, G * F], FDT, tag="agg")
            xd_own = pp.tile([P, G * F], HDT, tag="xdown")

            dinv_b = dinv_t[:].to_broadcast([P, G, F])

            def scale_to_table(src_tile, scope):
                with nc.named_scope(scope):
                    nc.vector.tensor_tensor(
                        out=xd_own[:].rearrange("p (g f) -> p g f", f=F),
                        in0=src_tile[:].rearrange("p (g f) -> p g f", f=F),
                        in1=dinv_b,
                        op=mybir.AluOpType.mult,
                    )
                    nc.sync.dma_start(out=cc_in[:], in_=xd_own[:])
                    nc.gpsimd.collective_compute(
                        "AllGather",
                        mybir.AluOpType.bypass,
                        replica_groups=[list(range(8))],
                        ins=[cc_in[:]],
                        outs=[cc_out[:]],
                    )
                    for qq in range(4):
                        nc.sync.dma_start(
                            out=table[qq * QROWS : (qq + 1) * QROWS, :F],
                            in_=cc_out[qq * QROWS : (qq + 1) * QROWS, :],
                        )

            def gather_layer(scope):
                with nc.named_scope(scope):
                    # self-loop contribution: agg starts at xd_own
                    nc.vector.tensor_copy(out=agg[:], in_=xd_own[:])
                    for (q, g, k0, kc, col0) in plan:
                        gt = gp.tile([P, KCAP, 4 * F], HDT, tag="g")
                        nc.gpsimd.dma_gather(
                            out_ap=gt[:, :kc, :],
                            in_ap=table[q * QROWS : (q + 1) * QROWS, :],
                            idxs_ap=offs_t[:, col0 : col0 + kc * P // 16],
                            num_idxs=kc * P,
                            num_idxs_reg=kc * P,
                            elem_size=4 * F,
                            queue_num=(q * G + g) % 4,
                        )
                        if kc == 1:
                            nc.vector.tensor_add(
                                out=agg[:, g * F : (g + 1) * F],
                                in0=agg[:, g * F : (g + 1) * F],
                                in1=gt[:, 0, :F],
                            )
                        else:
                            # fp16 pairs added into f32 (no fp16 accumulation)
                            h2 = kc // 2
                            h = (kc + 1) // 2
                            red = sp.tile([P, 4, F], FDT, tag="red")
                            nc.vector.tensor_add(
                                out=red[:, :h2, :],
                                in0=gt[:, 0 : 2 * h2 : 2, :F],
                                in1=gt[:, 1 : 2 * h2 : 2, :F],
                            )
                            if kc % 2:
                                nc.vector.tensor_copy(
                                    out=red[:, h2, :], in_=gt[:, kc - 1, :F]
                                )
                            if h == 1:
                                nc.vector.tensor_add(
                                    out=agg[:, g * F : (g + 1) * F],
                                    in0=agg[:, g * F : (g + 1) * F],
                                    in1=red[:, 0, :],
                                )
                            else:
                                red2 = sp.tile([P, F], FDT, tag="red2")
                                nc.vector.reduce_sum(
                                    out=red2[:],
                                    in_=red[:, :h, :].rearrange("p k f -> p f k"),
                                    axis=mybir.AxisListType.X,
                                )
                                nc.vector.tensor_add(
                                    out=agg[:, g * F : (g + 1) * F],
                                    in0=agg[:, g * F : (g + 1) * F],
                                    in1=red2[:],
                                )

            def layer_tail(W_t, bias_t, relu, dest, scope, W2_t=None, bias2_t=None):
                with nc.named_scope(scope):
                    nc.vector.tensor_tensor(
                        out=agg[:].rearrange("p (g f) -> p g f", f=F),
                        in0=agg[:].rearrange("p (g f) -> p g f", f=F),
                        in1=dinv_b,
                        op=mybir.AluOpType.mult,
                    )
                    for g in range(G):
                        ps1 = psp.tile([F, P], FDT, tag="ps1")
                        nc.tensor.matmul(
                            out=ps1[:], lhsT=agg[:, g * F : (g + 1) * F], rhs=ident[:],
                            start=True, stop=True,
                        )
                        s1 = sp.tile([F, P], FDT, tag="s1")
                        nc.vector.tensor_copy(out=s1[:], in_=ps1[:])
                        ps2 = psp.tile([F, P], FDT, tag="ps2")
                        nc.tensor.matmul(out=ps2[:], lhsT=W_t[:], rhs=s1[:], start=True, stop=True)
                        s2 = sp.tile([F, P], FDT, tag="s2")
                        if relu:
                            nc.scalar.activation(
                                out=s2[:], in_=ps2[:],
                                func=mybir.ActivationFunctionType.Relu,
                                bias=b0_t[:, :1] if bias_t is b0_t else bias_t[:, :1],
                                scale=1.0,
                            )
                        else:
                            nc.vector.tensor_scalar(
                                out=s2[:], in0=ps2[:], scalar1=bias_t[:, :1],
                                scalar2=None, op0=mybir.AluOpType.add,
                            )
                        if W2_t is not None:
                            ps3 = psp.tile([F, P], FDT, tag="ps3")
                            nc.tensor.matmul(out=ps3[:], lhsT=W2_t[:], rhs=s2[:], start=True, stop=True)
                            s2b = sp.tile([F, P], FDT, tag="s2b")
                            nc.vector.tensor_scalar(
                                out=s2b[:], in0=ps3[:], scalar1=bias2_t[:, :1],
                                scalar2=None, op0=mybir.AluOpType.add,
                            )
                            s2 = s2b
                        psb = psp.tile([P, F], FDT, tag="psb")
                        nc.tensor.matmul(
                            out=psb[:], lhsT=s2[:], rhs=ident[:F, :F], start=True, stop=True
                        )
                        nc.vector.tensor_copy(out=dest[:, g * F : (g + 1) * F], in_=psb[:])

            scale_to_table(xcur, "table0")
            gather_layer("gather0")
            layer_tail(w0_t, b0_t, relu=True, dest=xcur, scope="tail0")
            scale_to_table(xcur, "table1")
            gather_layer("gather1")
            outt = pp.tile([P, G * F], FDT, tag="outt")
            layer_tail(
                w1_t, b1_t, relu=True, dest=outt, scope="tail1", W2_t=wf_t, bias2_t=bf_t
            )
            nc.sync.dma_start(out=out_own[:], in_=outt[:])

    nc.compile()
    return nc


_CACHE = {}


def kernel(x, edge_index, W0, b0, W1, b1, Wf, bf):
    x = np.asarray(x, dtype=np.float32)
    edge_index = np.asarray(edge_index)
    plan, totc, dinv, perms, offs_all = _build_plan_and_offsets(edge_index)

    key = ("prog", totc, len(plan))
    if key not in _CACHE:
        _CACHE[key] = _build_program(plan, totc)
    nc = _CACHE[key]

    in_maps = []
    rr = np.arange(NPC)
    pp_, gg = rr % P, rr // P
    for c in range(8):
        perm = perms[c]
        xo = np.zeros((P, G, F), dtype=np.float32)
        dv = np.zeros((P, G), dtype=np.float32)
        xo[pp_, gg, :] = x[perm]
        dv[pp_, gg] = dinv[perm]
        in_maps.append(
            {
                "x_own": xo.reshape(P, G * F),
                "dinv_own": dv,
                "offs": np.tile(offs_all[c], (8, 1)).astype(np.int16),
                "W0": np.asarray(W0, np.float32),
                "W1": np.asarray(W1, np.float32),
                "Wf": np.asarray(Wf, np.float32),
                "b0": np.asarray(b0, np.float32).reshape(F, 1),
                "b1": np.asarray(b1, np.float32).reshape(F, 1),
                "bf": np.asarray(bf, np.float32).reshape(F, 1),
            }
        )

    res = run_bass_kernel_spmd(nc, in_maps, list(range(8)))
    kernel._last_results = res

    out = np.zeros((N, F), dtype=np.float32)
    for c in range(8):
        oo = res.results[c]["out_own"].reshape(P, G, F)
        out[perms[c]] = oo[pp_, gg, :]
    return out



# revision 2
# speedup vs baseline: 2.1001x; 2.1001x over previous
"""Trainium2 Bass kernel for a 2-layer GCN (nn_CorrelationGNN) — v2.

Math (reference):
    src,dst = edges + self loops;  deg over dst;  dinv = deg^-1/2
    h1 = relu(S @ (x @ W0) + b0),  S = D^-1/2 (A+I) D^-1/2
    h2 = relu(S @ (h1 @ W1) + b1)
    out = h2 @ Wf + bf
Factorization: S @ (h W) = dinv_d * Agg(dinv_s * h) @ W.

v2 design (vs the per-dst-aligned v1):
  - fp16 feature table = the AllGather output itself, viewed as
    [25088 rows, 128 lanes] (row = 4 nodes x 32 feats = 256B) -> single
    int16-addressable window, no quarter splits, no strided table writes.
  - Edge tokens are a flat stream sorted by (dst column g, src band b),
    gathered in 8192-idx dma_gather chunks (31/layer instead of 702).
  - Reduction on the Tensor engine: per 128-token tile, PSUM[g] +=
    gathered[:, s, b*32:(b+1)*32].T @ assign (assign = fp8 0/1 token->dst
    matrix, precomputed on host, streamed from HBM). Output lands
    feat-major [32, 128] which feeds the W matmul without a transpose.
  - Self loops are ordinary tokens. Padding tokens get an all-zero
    assign row, so any gather idx works for them.
"""

import numpy as np
import ml_dtypes

import concourse.bass as bass  # noqa: F401
import concourse.bacc as bacc
import concourse.mybir as mybir
from concourse.tile import TileContext
from concourse.bass_utils import run_bass_kernel_spmd

P = 128
N = 100000
F = 32
NPC = 12500          # real nodes per core
G = 98               # dst columns per core
NPCP = P * G         # padded nodes per core = 12544
TROWS = 8 * NPCP     # global table rows (nodes) = 100352
PROWS = TROWS // 4   # packed 256B rows = 25088 (int16-addressable)
CH = 1024            # tokens per dma_gather chunk (hard ucode/ring cap)
SLOTS = CH // P      # 8 matmul tiles per chunk
ABATCH = 8           # chunks per assign-matrix DMA batch
FDT = mybir.dt.float32
HDT = mybir.dt.float16
ADT = mybir.dt.float8e4
NP_F8 = ml_dtypes.float8_e4m3fn


def _build_plan(edge_index):
    src = np.asarray(edge_index[0], dtype=np.int64)
    dst = np.asarray(edge_index[1], dtype=np.int64)
    loops = np.arange(N, dtype=np.int64)
    srcA = np.concatenate([src, loops])
    dstA = np.concatenate([dst, loops])

    deg = np.bincount(dstA, minlength=N).astype(np.float64)
    dinv = (1.0 / np.sqrt(deg)).astype(np.float32)

    # node -> (core, rank); degree-sorted ranks keep per-column token
    # counts similar across cores (shrinks the cross-core max padding)
    rank = np.empty(N, dtype=np.int64)
    perms = []
    for c in range(8):
        nodes = np.arange(c * NPC, (c + 1) * NPC)
        order = np.argsort(-deg[nodes], kind="stable")
        perm = nodes[order]
        perms.append(perm)
        rank[perm] = np.arange(NPC)
    node_core = np.arange(N) // NPC
    trow = node_core * NPCP + (rank % P) * G + (rank // P)
    tok_of_node = (trow // 4).astype(np.int64)   # packed row
    band_of_node = (trow % 4).astype(np.int64)   # 32-lane band in row

    # per-core edge lists sorted by (dst column, src band)
    ecore = dstA // NPC
    per_core = []
    cnt = np.zeros((8, G, 4), np.int64)
    for c in range(8):
        m = ecore == c
        s, d = srcA[m], dstA[m]
        g = rank[d] // P
        p = rank[d] % P
        b = band_of_node[s]
        order = np.lexsort((b, g))
        g, p, b, s = g[order], p[order], b[order], s[order]
        per_core.append((g, p, b, tok_of_node[s]))
        np.add.at(cnt[c], (g, b), 1)

    # shared tile counts: T[g,b] = max over cores of ceil(cnt/128)
    T = np.maximum.reduce([
        np.ceil(cnt[c] / P).astype(np.int64) for c in range(8)
    ])
    nslots = int(T.sum())
    nchunks = -(-nslots * P // CH)
    # pad the stream to a whole number of assign DMA batches
    ncols = (-(-nchunks // ABATCH)) * ABATCH * CH

    # plan: per slot (g, band, start, stop) in (g, b, k) order
    plan = []
    for g in range(G):
        ngs = int(T[g].sum())
        k0 = 0
        for b in range(4):
            for _ in range(int(T[g, b])):
                plan.append((g, b, k0 == 0, k0 == ngs - 1))
                k0 += 1
    assert len(plan) == nslots

    # slot base offsets in the token stream, per (g, b)
    base_gb = np.zeros((G, 4), np.int64)
    pos = 0
    for g in range(G):
        for b in range(4):
            base_gb[g, b] = pos
            pos += int(T[g, b]) * P

    # per-core offset + assign arrays
    rng = np.random.default_rng(12345)
    offs_all, assign_all = [], []
    for c in range(8):
        g, p, b, tok = per_core[c]
        # position of each edge within its (g,b) segment
        keys = g * 4 + b
        # edges are sorted by key; positions restart at key changes
        starts = np.flatnonzero(np.diff(keys, prepend=-1))
        segpos = np.arange(len(g)) - np.repeat(starts, np.diff(
            np.append(starts, len(g))))
        stream_pos = base_gb[g, b] + segpos

        offs = rng.integers(0, PROWS, size=ncols).astype(np.int16)
        offs[stream_pos] = tok.astype(np.int16)
        # wrap [16, ncols//16]: idx i at [i%16, i//16], replicate to 128
        offs16 = offs.reshape(ncols // 16, 16).T.copy()
        offs_all.append(np.tile(offs16, (8, 1)))

        asg = np.zeros((P, ncols), dtype=np.uint8)
        # token at stream pos i: partition i%128, col (i//128)*128 + dst_p
        asg[stream_pos % P, (stream_pos // P) * P + p] = 1
        assign_all.append(asg)

    one_f8 = np.asarray(1.0, dtype=NP_F8).view(np.uint8)
    for c in range(8):
        a = assign_all[c]
        a *= one_f8  # 0 stays 0 (fp8 zero), 1 -> fp8 1.0 bit pattern
        assign_all[c] = a.view(NP_F8)

    return plan, nchunks, dinv, perms, offs_all, assign_all


def _build_program(plan, nchunks):
    nslots = len(plan)
    ncols = (-(-nchunks // ABATCH)) * ABATCH * CH
    nc = bacc.Bacc(
        "TRN2", target_bir_lowering=False, debug=False, num_devices=8,
        num_swdge_queues=4,
    )
    x_own = nc.declare_dram_parameter("x_own", [P, G * F], FDT, isOutput=False)
    dinvF = nc.declare_dram_parameter("dinvF", [P, G * F], FDT, isOutput=False)
    dinvT = nc.declare_dram_parameter("dinvT", [F, NPCP], FDT, isOutput=False)
    offs = nc.declare_dram_parameter(
        "offs", [P, ncols // 16], mybir.dt.int16, isOutput=False)
    assign = nc.declare_dram_parameter("assign", [P, ncols], ADT, isOutput=False)
    W0 = nc.declare_dram_parameter("W0", [F, F], FDT, isOutput=False)
    W1 = nc.declare_dram_parameter("W1", [F, F], FDT, isOutput=False)
    Wf = nc.declare_dram_parameter("Wf", [F, F], FDT, isOutput=False)
    b0 = nc.declare_dram_parameter("b0", [F, 1], FDT, isOutput=False)
    b1 = nc.declare_dram_parameter("b1", [F, 1], FDT, isOutput=False)
    bf = nc.declare_dram_parameter("bf", [F, 1], FDT, isOutput=False)
    out_own = nc.declare_dram_parameter("out_own", [P, G * F], FDT, isOutput=True)

    cc_in = nc.dram_tensor("cc_in", [NPCP, F], HDT)
    cc_out = nc.dram_tensor("cc_out", [TROWS, F], HDT, addr_space="Shared")

    from concourse.masks import make_identity

    with TileContext(nc) as tc:
        with (
            tc.tile_pool(name="persist", bufs=1) as pp,
            tc.tile_pool(name="gpool", bufs=3) as gp,
            tc.tile_pool(name="apool", bufs=3) as ap_,
            tc.tile_pool(name="spool", bufs=4) as sp,
            tc.tile_pool(name="psacc", bufs=4, space="PSUM") as psa,
            tc.tile_pool(name="pstail", bufs=1, space="PSUM") as pst_,
        ):
            offs_t = pp.tile([P, ncols // 16], mybir.dt.int16)
            nc.sync.dma_start(out=offs_t[:], in_=offs[:])
            dinvF_t = pp.tile([P, G * F], FDT)
            nc.sync.dma_start(out=dinvF_t[:], in_=dinvF[:])
            dinvT_t = pp.tile([F, NPCP], FDT)
            nc.sync.dma_start(out=dinvT_t[:], in_=dinvT[:])
            w0_t = pp.tile([F, F], FDT)
            nc.sync.dma_start(out=w0_t[:], in_=W0[:])
            w1_t = pp.tile([F, F], FDT)
            nc.sync.dma_start(out=w1_t[:], in_=W1[:])
            wf_t = pp.tile([F, F], FDT)
            nc.sync.dma_start(out=wf_t[:], in_=Wf[:])
            b0_t = pp.tile([F, 1], FDT)
            nc.sync.dma_start(out=b0_t[:], in_=b0[:])
            b1_t = pp.tile([F, 1], FDT)
            nc.sync.dma_start(out=b1_t[:], in_=b1[:])
            bf_t = pp.tile([F, 1], FDT)
            nc.sync.dma_start(out=bf_t[:], in_=bf[:])
            ident = pp.tile([P, P], FDT)
            make_identity(nc, ident[:])

            xd = pp.tile([P, G * F], HDT, tag="xd")
            outt = pp.tile([P, G * F], FDT, tag="outt")

            xf = gp.tile([P, G * F], FDT, tag="g")
            nc.sync.dma_start(out=xf[:], in_=x_own[:])
            nc.vector.tensor_tensor(
                out=xd[:], in0=xf[:], in1=dinvF_t[:], op=mybir.AluOpType.mult)

            table_ap = cc_out[:].rearrange("(r x) f -> r (x f)", x=4)

            def emit_tail(layer, g, acc):
                s1 = sp.tile([F, P], FDT, tag="s1")
                nc.vector.tensor_tensor(
                    out=s1[:], in0=acc[:], in1=dinvT_t[:, g * P:(g + 1) * P],
                    op=mybir.AluOpType.mult)
                W_t = w0_t if layer == 0 else w1_t
                bias_t = b0_t if layer == 0 else b1_t
                ps2 = pst_.tile([F, P], FDT, tag="ps2")
                nc.tensor.matmul(out=ps2[:], lhsT=W_t[:], rhs=s1[:],
                                 start=True, stop=True)
                s2 = sp.tile([F, P], FDT, tag="s2")
                nc.scalar.activation(
                    out=s2[:], in_=ps2[:],
                    func=mybir.ActivationFunctionType.Relu,
                    bias=bias_t[:, :1], scale=1.0)
                if layer == 0:
                    ps4 = pst_.tile([P, F], FDT, tag="ps4")
                    nc.tensor.matmul(out=ps4[:], lhsT=s2[:], rhs=ident[:F, :F],
                                     start=True, stop=True)
                    nc.vector.tensor_tensor(
                        out=xd[:, g * F:(g + 1) * F],
                        in0=ps4[:], in1=dinvF_t[:, g * F:(g + 1) * F],
                        op=mybir.AluOpType.mult)
                else:
                    ps3 = pst_.tile([F, P], FDT, tag="ps3")
                    nc.tensor.matmul(out=ps3[:], lhsT=wf_t[:], rhs=s2[:],
                                     start=True, stop=True)
                    s3 = sp.tile([F, P], FDT, tag="s3")
                    nc.vector.tensor_scalar(
                        out=s3[:], in0=ps3[:], scalar1=bf_t[:, :1],
                        scalar2=None, op0=mybir.AluOpType.add)
                    ps4 = pst_.tile([P, F], FDT, tag="ps4")
                    nc.tensor.matmul(out=ps4[:], lhsT=s3[:], rhs=ident[:F, :F],
                                     start=True, stop=True)
                    nc.vector.tensor_copy(
                        out=outt[:, g * F:(g + 1) * F], in_=ps4[:])

            for layer in (0, 1):
                with nc.named_scope(f"table{layer}"):
                    nc.sync.dma_start(out=cc_in[:], in_=xd[:])
                    nc.gpsimd.collective_compute(
                        "AllGather",
                        mybir.AluOpType.bypass,
                        replica_groups=[list(range(8))],
                        ins=[cc_in[:]],
                        outs=[cc_out[:]],
                    )
                with nc.named_scope(f"agg{layer}"):
                    acc = None
                    for c in range(nchunks):
                        if c % ABATCH == 0:
                            at = ap_.tile([P, ABATCH * CH], ADT, tag="a")
                            nc.sync.dma_start(
                                out=at[:],
                                in_=assign[:, c * CH:(c + ABATCH) * CH])
                        ao = (c % ABATCH) * CH
                        gt = gp.tile([P, SLOTS, 4 * F], HDT, tag="g")
                        nc.gpsimd.dma_gather(
                            out_ap=gt[:],
                            in_ap=table_ap,
                            idxs_ap=offs_t[:, c * (CH // 16):(c + 1) * (CH // 16)],
                            num_idxs=CH,
                            num_idxs_reg=CH,
                            elem_size=4 * F,
                            queue_num=c % 4,
                        )
                        for sl in range(SLOTS):
                            s = c * SLOTS + sl
                            if s >= nslots:
                                break
                            g, b, st, sp_flag = plan[s]
                            if st:
                                acc = psa.tile([F, P], FDT, tag="acc")
                            nc.tensor.matmul(
                                out=acc[:],
                                lhsT=gt[:, sl, b * F:(b + 1) * F],
                                rhs=at[:, ao + sl * P:ao + (sl + 1) * P],
                                start=st, stop=sp_flag)
                            if sp_flag:
                                emit_tail(layer, g, acc)

            nc.sync.dma_start(out=out_own[:], in_=outt[:])

    nc.compile()
    return nc


_CACHE = {}


def kernel(x, edge_index, W0, b0, W1, b1, Wf, bf):
    x = np.asarray(x, dtype=np.float32)
    edge_index = np.asarray(edge_index)
    plan, nchunks, dinv, perms, offs_all, assign_all = _build_plan(edge_index)

    key = ("prog_v2", nchunks, len(plan), tuple(p[0] for p in plan[::97]))
    if key not in _CACHE:
        _CACHE[key] = _build_program(plan, nchunks)
    nc = _CACHE[key]

    in_maps = []
    rr = np.arange(NPC)
    pp_, gg = rr % P, rr // P
    for c in range(8):
        perm = perms[c]
        xo = np.zeros((P, G, F), dtype=np.float32)
        dv = np.zeros((P, G), dtype=np.float32)
        xo[pp_, gg, :] = x[perm]
        dv[pp_, gg] = dinv[perm]
        dinvF_h = np.repeat(dv[:, :, None], F, axis=2).reshape(P, G * F)
        # dinvT[f, g*128+p] = dinv[p, g]
        dinvT_h = np.broadcast_to(
            dv.T.reshape(1, G * P), (F, G * P)).copy()
        in_maps.append(
            {
                "x_own": xo.reshape(P, G * F),
                "dinvF": dinvF_h.astype(np.float32),
                "dinvT": dinvT_h.astype(np.float32),
                "offs": offs_all[c],
                "assign": assign_all[c],
                "W0": np.asarray(W0, np.float32),
                "W1": np.asarray(W1, np.float32),
                "Wf": np.asarray(Wf, np.float32),
                "b0": np.asarray(b0, np.float32).reshape(F, 1),
                "b1": np.asarray(b1, np.float32).reshape(F, 1),
                "bf": np.asarray(bf, np.float32).reshape(F, 1),
            }
        )

    res = run_bass_kernel_spmd(nc, in_maps, list(range(8)))
    kernel._last_results = res

    out = np.zeros((N, F), dtype=np.float32)
    for c in range(8):
        oo = res.results[c]["out_own"].reshape(P, G, F)
        out[perms[c]] = oo[pp_, gg, :]
    return out
